# revision 9
# baseline (speedup 1.0000x reference)
"""Trainium2 Bass kernel for nn_AdaptivePoolingClassifier (8 NeuronCores).

Math: the reference MLP is linear up to its single ReLU, so W1..W3 fold
into one 128x128 matrix on the host:
    h   = relu(x @ Wc^T + bc)       Wc = W3 W2 W1 ; bc = W3(W2 b1+b2)+b3
    p   = h @ W4^T + b4
    out = sum_n p * softmax(alpha*p, axis=1)

Device computes pt = h @ (diag(alpha) W4)^T = alpha*(p - b4) for every
row (rows sharded 8 ways) and streams pt back to DRAM; the host finishes
the softmax pooling in f64 (num/den sums over rows) exactly as it
already finishes the fold / bias algebra.  The softmax weights are
invariant to the per-column constant alpha*b4 shift.

Device schedule (v5) — measured-engine-rate driven:
  - x host-transposed to [128(feat), rows] fp8e4 (halves HBM traffic;
    bf16 weights, mixed-dtype matmul, rel err ~1.8e-3).
  - All x DMAs issued UP FRONT from two queues (sync: cst+g0, gpsimd:
    g1..g4) into persistent SBUF tiles (24.5KB/partition) - no buffer
    recycling, no mid-kernel issue cost on busy queues.
  - PE warms up on garbage SBUF from t~1.5us (no data deps) so the
    DVFS ramp overlaps the initial DMA instead of following it.
  - Steady tiles of 1536 cols: 3x512 L1 matmuls (wct stationary) into
    a 3-bank psum tile, one whole-tile relu (ACT / DVE alternating,
    9:7 split matched to 1.33 vs 0.9 GHz effective rates) -> bf16 h,
    then 12 L4 chunk matmuls (h-chunk stationary [128,128], w4at
    moving 5 cols) whose ldweights pipeline back-to-back (~35ns/chunk
    measured dense vs ~100ns exposed).
  - pt accumulates in psum [128, 48, 5] batches; batch ends staggered
    (48, 96, 144, 180, 196) so copies+DMAs overlap the PE stream.
"""

import numpy as np
import ml_dtypes

from concourse import bacc, mybir, tile
from concourse.bass_utils import run_bass_kernel_spmd

N_CORES = 8
N_ROWS = 200000
F = 128
OUT = 5

ROWS_PAD = 200704            # 8 * 25088
RPC = ROWS_PAD // N_CORES    # rows per core = 25088
T0 = 512                     # prologue tile
TILE = 1536                  # steady tile (3 psum banks)
N_TILES = (RPC - T0) // TILE  # 16
CHUNK = 128
N_CHUNKS = RPC // CHUNK      # 196
SLOTS = 48                   # max pt chunks per psum batch
BATCH_ENDS = (48, 96, 144, 184, 196)
BATCH_ENGINE = ("act", "dve", "act", "dve", "dve")
# x DMA groups: xg0 covers prologue+1 tile (hides xg1 transfer latency)
XG_SIZES = (T0 + TILE, 4 * TILE, 4 * TILE, 4 * TILE, 3 * TILE)
# every relu is split: ACT gets cols [0:RELU_A), DVE the rest
RELU_A = 768

F32 = mybir.dt.float32
BF16 = mybir.dt.bfloat16
FP8 = mybir.dt.float8e4
AF = mybir.ActivationFunctionType
ALU = mybir.AluOpType


def build_bass(has_bias=False):
    nc = bacc.Bacc()

    CONST_COLS = (F + OUT + 1) if has_bias else (F + OUT)
    cst_ext = nc.declare_dram_parameter(
        "cst", [F, CONST_COLS], BF16, isOutput=False
    )
    xg_ext = [
        nc.declare_dram_parameter(f"xg{i}", [F, w], FP8, isOutput=False)
        for i, w in enumerate(XG_SIZES)
    ]
    pt_ext = nc.declare_dram_parameter(
        "pt", [F, N_CHUNKS, OUT], F32, isOutput=True
    )

    with tile.TileContext(nc) as tc:
        with (
            tc.tile_pool(name="scratch", bufs=1) as scratch,
            tc.tile_pool(name="xin", bufs=1) as xin,
            tc.tile_pool(name="hbuf", bufs=4) as hbuf,
            tc.tile_pool(name="ptb", bufs=2) as ptb,
            tc.tile_pool(name="ps_h", bufs=2, space="PSUM") as ps_h,
            tc.tile_pool(name="ps_p", bufs=2, space="PSUM") as ps_p,
        ):
            # --- junk memsets first on gpsimd (earliest user queue) ------
            junk_w = scratch.tile([F, CHUNK], BF16)
            junk_x = scratch.tile([F, 256], FP8)
            junk_o = scratch.tile([F, 8], BF16)
            nc.gpsimd.memset(junk_w[:], 1.0)
            nc.gpsimd.memset(junk_x[:], 1.0)

            # --- upfront DMA issue, ALL on sync: per-queue FIFO means
            # cst/xg0 complete first at full aggregate DMA bandwidth ------
            cstt = scratch.tile([F, CONST_COLS], BF16)
            xg = [xin.tile([F, w], FP8, tag=f"xg{i}", name=f"xg{i}")
                  for i, w in enumerate(XG_SIZES)]
            nc.sync.dma_start(out=cstt[:], in_=cst_ext[:])
            for i in (0, 1, 2, 3, 4):
                nc.sync.dma_start(out=xg[i][:], in_=xg_ext[i][:])

            wct = cstt[:, :F]
            w4at = cstt[:, F : F + OUT]
            bc = None
            if has_bias:
                bc = scratch.tile([F, 1], F32)

            # --- engine warmups (gated only on the vector memsets) -------
            pw = ps_h.tile([F, TILE], F32, tag="hp", name="pw")
            # preload ACT relu table during the DMA wait
            nc.scalar.activation(junk_o[:], junk_w[:, :8], AF.Relu)
            for r in range(6):
                nc.tensor.matmul(
                    pw[:, :256], junk_w, junk_x[:], start=True, stop=True,
                    skip_group_check=True,
                )
            if has_bias:
                nc.vector.tensor_copy(bc[:], cstt[:, F + OUT : F + OUT + 1])

            state = {"chunk": 0, "pp": None, "bstart": 0, "bi": 0}
            hbufs = []  # per tile: (htile, n_chunks)

            def act_relu(dst, src):
                if has_bias:
                    nc.scalar.activation(dst, src, AF.Relu, bias=bc[:], scale=1.0)
                else:
                    nc.scalar.activation(dst, src, AF.Relu)

            def dve_relu(dst, src):
                if has_bias:
                    nc.vector.tensor_scalar(dst, src, bc[:], 0.0, ALU.add, ALU.max)
                else:
                    nc.vector.tensor_scalar_max(dst, src, 0.0)

            def do_l1(ti, rhs, width):
                hp = ps_h.tile([F, TILE], F32, tag="hp", name="hp")
                for c in range(0, width, 512):
                    nc.tensor.matmul(
                        hp[:, c : c + 512], wct, rhs[:, c : c + 512],
                        start=True, stop=True,
                    )
                ht = hbuf.tile([F, TILE], BF16, tag="ht")
                a = min(RELU_A, width // 2) if width < TILE else RELU_A
                act_relu(ht[:, :a], hp[:, :a])
                dve_relu(ht[:, a:width], hp[:, a:width])
                hbufs.append((ht, width // CHUNK))

            def do_l4(ti):
                ht, n_ch = hbufs[ti]
                for j in range(n_ch):
                    c = state["chunk"]
                    s = c - state["bstart"]
                    if s == 0:
                        state["pp"] = ps_p.tile(
                            [F, SLOTS, OUT], F32, tag="pp", name="pp"
                        )
                    nc.tensor.matmul(
                        state["pp"][:, s, :],
                        ht[:, j * CHUNK : (j + 1) * CHUNK], w4at,
                        start=True, stop=True,
                    )
                    state["chunk"] = c + 1
                    if state["chunk"] in BATCH_ENDS:
                        c0 = state["bstart"]
                        n = state["chunk"] - c0
                        bi = state["bi"]
                        pts = ptb.tile([F, SLOTS, OUT], F32, tag="pts")
                        if BATCH_ENGINE[bi] == "dve":
                            nc.vector.tensor_copy(
                                pts[:, :n, :], state["pp"][:, :n, :]
                            )
                        else:
                            nc.scalar.activation(
                                pts[:, :n, :], state["pp"][:, :n, :], AF.Copy,
                            )
                        nc.sync.dma_start(
                            out=pt_ext[:, c0 : state["chunk"], :],
                            in_=pts[:, :n, :],
                        )
                        state["bstart"] = state["chunk"]
                        state["bi"] = bi + 1

            # prologue tile (hbufs[0]); steady tile t -> hbufs[t+1]
            bounds = []
            b = 0
            for w in XG_SIZES:
                bounds.append((b, b + w))
                b += w

            def xg_slice(c0, width):
                for g, (lo, hi) in enumerate(bounds):
                    if lo <= c0 and c0 + width <= hi:
                        return xg[g][:, c0 - lo : c0 - lo + width]
                raise AssertionError(f"tile [{c0}, {c0+width}) crosses groups")

            do_l1(-1, xg_slice(0, T0), T0)
            for t in range(N_TILES):
                do_l1(t, xg_slice(T0 + t * TILE, TILE), TILE)
                if t >= 1:
                    do_l4(t - 1)   # lag-2: chunks of tile t-2
            do_l4(N_TILES - 1)
            do_l4(N_TILES)

    nc.finalize()
    return nc


_CACHED = {}
TRACE = False
LAST = {}


def kernel(x, W1, b1, W2, b2, W3, b3, W4, b4, alpha):
    f64 = np.float64
    x2 = np.asarray(x, np.float32).reshape(N_ROWS, F)
    W1, b1, W2, b2, W3, b3, W4, b4, alpha = [
        np.asarray(a, f64) for a in (W1, b1, W2, b2, W3, b3, W4, b4, alpha)
    ]

    # fold the linear layers (exact in f64)
    Wc = W3 @ W2 @ W1
    bc = W3 @ (W2 @ b1 + b2) + b3
    alpha_safe = np.where(np.abs(alpha) < 1e-12, 1e-12, alpha)
    W4a = alpha_safe[:, None] * W4

    # pad rows to 8*25088 with zeros; pad rows dropped after the gather
    n_pad = ROWS_PAD - N_ROWS
    xp = np.concatenate([x2, np.zeros((n_pad, F), np.float32)], axis=0)
    xT = np.ascontiguousarray(xp.T).astype(ml_dtypes.float8_e4m3fn)

    has_bias = bool(np.any(bc != 0.0))
    key = ("nc", has_bias)
    if key not in _CACHED:
        _CACHED[key] = build_bass(has_bias)
    nc = _CACHED[key]

    wct_np = np.ascontiguousarray(Wc.T).astype(ml_dtypes.bfloat16)
    w4at_np = np.ascontiguousarray(W4a.T).astype(ml_dtypes.bfloat16)
    parts_list = [wct_np, w4at_np]
    if has_bias:
        parts_list.append(
            bc.reshape(F, 1).astype(np.float32).astype(ml_dtypes.bfloat16)
        )
    consts_np = np.ascontiguousarray(np.concatenate(parts_list, axis=1))

    bounds = np.cumsum((0,) + XG_SIZES)
    in_maps = []
    for c in range(N_CORES):
        shard = xT[:, c * RPC : (c + 1) * RPC]
        m = {"cst": consts_np}
        for i in range(len(XG_SIZES)):
            m[f"xg{i}"] = np.ascontiguousarray(shard[:, bounds[i] : bounds[i + 1]])
        in_maps.append(m)

    res = run_bass_kernel_spmd(
        nc, in_maps, core_ids=list(range(N_CORES)), trace=TRACE
    )
    LAST["res"] = res

    # gather pt: per core [F(part=row-in-chunk), N_CHUNKS, OUT]
    pts = np.stack([np.asarray(r["pt"], np.float32) for r in res.results])
    # rows order: (core, chunk, partition)
    pt = pts.transpose(0, 2, 1, 3).reshape(ROWS_PAD, OUT).astype(f64)
    pt = pt[:N_ROWS]

    # host softmax pooling in f64:  out_o = sum pt*e^pt / (alpha*sum e^pt) + b4
    m = pt.max(axis=0)
    e = np.exp(pt - m)
    den = e.sum(axis=0)
    num = (pt * e).sum(axis=0)
    out = num / (alpha_safe * den) + b4
    return out[None, :].astype(np.float32)


# revision 10
# speedup vs baseline: 1.2234x; 1.2234x over previous
"""Trainium2 Bass kernel for nn_AdaptivePoolingClassifier (8 NeuronCores).

Math: the reference MLP is linear up to its single ReLU, so W1..W3 fold
into one 128x128 matrix on the host:
    h   = relu(x @ Wc^T + bc)       Wc = W3 W2 W1 ; bc = W3(W2 b1+b2)+b3
    p   = h @ W4^T + b4
    out = sum_n p * softmax(alpha*p, axis=1)

Device computes pt = h @ (diag(alpha) W4)^T = alpha*(p - b4) for every
row (rows sharded 8 ways) and streams pt back to DRAM; the host finishes
the softmax pooling in f64 (num/den sums over rows) exactly as it
already finishes the fold / bias algebra.  The softmax weights are
invariant to the per-column constant alpha*b4 shift.

Device schedule (v5) — measured-engine-rate driven:
  - x host-transposed to [128(feat), rows] fp8e4 (halves HBM traffic;
    bf16 weights, mixed-dtype matmul, rel err ~1.8e-3).
  - All x DMAs issued UP FRONT from two queues (sync: cst+g0, gpsimd:
    g1..g4) into persistent SBUF tiles (24.5KB/partition) - no buffer
    recycling, no mid-kernel issue cost on busy queues.
  - PE warms up on garbage SBUF from t~1.5us (no data deps) so the
    DVFS ramp overlaps the initial DMA instead of following it.
  - Steady tiles of 1536 cols: 3x512 L1 matmuls (wct stationary) into
    a 3-bank psum tile, one whole-tile relu (ACT / DVE alternating,
    9:7 split matched to 1.33 vs 0.9 GHz effective rates) -> bf16 h,
    then 12 L4 chunk matmuls (h-chunk stationary [128,128], w4at
    moving 5 cols) whose ldweights pipeline back-to-back (~35ns/chunk
    measured dense vs ~100ns exposed).
  - pt accumulates in psum [128, 48, 5] batches; batch ends staggered
    (48, 96, 144, 180, 196) so copies+DMAs overlap the PE stream.
"""

import numpy as np
import ml_dtypes

from concourse import bacc, mybir, tile
from concourse.bass_utils import run_bass_kernel_spmd

N_CORES = 8
N_ROWS = 200000
F = 128
OUT = 5

ROWS_PAD = 200704            # 8 * 25088
RPC = ROWS_PAD // N_CORES    # rows per core = 25088
T0 = 512                     # prologue tile
TILE = 1024                  # steady tile (2 psum banks)
N_TILES = (RPC - T0) // TILE  # 24
CHUNK = 128
N_CHUNKS = RPC // CHUNK      # 196
SLOTS = 48                   # max pt chunks per psum batch
BATCH_ENDS = (48, 96, 144, 188, 196)
BATCH_ENGINE = ("act", "dve", "act", "dve", "dve")
# ramping x DMA groups, all FIFO on sync: early groups small so the
# first tiles never starve, later groups big to bound issue count
XG_SIZES = (T0 + TILE, TILE, 2 * TILE, 3 * TILE, 4 * TILE, 6 * TILE,
            7 * TILE)
# tiles on the DVE relu path (rest on ACT); last two split across both
DVE_TILES = frozenset((1, 3, 5, 7, 9, 11, 13, 15, 17, 19, 21))
SPLIT_TILES = frozenset((22, 23))

F32 = mybir.dt.float32
BF16 = mybir.dt.bfloat16
FP8 = mybir.dt.float8e4
AF = mybir.ActivationFunctionType
ALU = mybir.AluOpType


def build_bass(has_bias=False):
    nc = bacc.Bacc()

    CONST_COLS = (F + OUT + 1) if has_bias else (F + OUT)
    cst_ext = nc.declare_dram_parameter(
        "cst", [F, CONST_COLS], BF16, isOutput=False
    )
    xg_ext = [
        nc.declare_dram_parameter(f"xg{i}", [F, w], FP8, isOutput=False)
        for i, w in enumerate(XG_SIZES)
    ]
    pt_ext = nc.declare_dram_parameter(
        "pt", [F, N_CHUNKS, OUT], F32, isOutput=True
    )

    with tile.TileContext(nc) as tc:
        with (
            tc.tile_pool(name="scratch", bufs=1) as scratch,
            tc.tile_pool(name="xin", bufs=1) as xin,
            tc.tile_pool(name="hbuf", bufs=4) as hbuf,
            tc.tile_pool(name="ptb", bufs=2) as ptb,
            tc.tile_pool(name="ps_h", bufs=3, space="PSUM") as ps_h,
            tc.tile_pool(name="ps_p", bufs=2, space="PSUM") as ps_p,
        ):
            # --- junk memsets first on gpsimd (earliest user queue) ------
            junk_w = scratch.tile([F, CHUNK], BF16)
            junk_x = scratch.tile([F, 256], FP8)
            junk_o = scratch.tile([F, 8], BF16)
            nc.gpsimd.memset(junk_w[:], 1.0)
            nc.gpsimd.memset(junk_x[:], 1.0)

            # --- upfront DMA issue, ALL on sync: per-queue FIFO means
            # cst/xg0 complete first at full aggregate DMA bandwidth ------
            cstt = scratch.tile([F, CONST_COLS], BF16)
            xg = [xin.tile([F, w], FP8, tag=f"xg{i}", name=f"xg{i}")
                  for i, w in enumerate(XG_SIZES)]
            nc.sync.dma_start(out=cstt[:], in_=cst_ext[:])
            for i in range(len(XG_SIZES)):
                nc.sync.dma_start(out=xg[i][:], in_=xg_ext[i][:])

            wct = cstt[:, :F]
            w4at = cstt[:, F : F + OUT]
            bc = None
            if has_bias:
                bc = scratch.tile([F, 1], F32)

            # --- engine warmups (gated only on the vector memsets) -------
            pw = ps_h.tile([F, TILE], F32, tag="hp", name="pw")
            # preload ACT relu table during the DMA wait
            nc.scalar.activation(junk_o[:], junk_w[:, :8], AF.Relu)
            for r in range(6):
                nc.tensor.matmul(
                    pw[:, :256], junk_w, junk_x[:], start=True, stop=True,
                    skip_group_check=True,
                )
            if has_bias:
                nc.vector.tensor_copy(bc[:], cstt[:, F + OUT : F + OUT + 1])

            state = {"chunk": 0, "pp": None, "bstart": 0, "bi": 0}
            hbufs = []  # per tile: (htile, n_chunks)

            def act_relu(dst, src):
                if has_bias:
                    nc.scalar.activation(dst, src, AF.Relu, bias=bc[:], scale=1.0)
                else:
                    nc.scalar.activation(dst, src, AF.Relu)

            def dve_relu(dst, src):
                if has_bias:
                    nc.vector.tensor_scalar(dst, src, bc[:], 0.0, ALU.add, ALU.max)
                else:
                    nc.vector.tensor_scalar_max(dst, src, 0.0)

            def do_l1(ti, rhs, width):
                hp = ps_h.tile([F, TILE], F32, tag="hp", name="hp")
                for c in range(0, width, 512):
                    nc.tensor.matmul(
                        hp[:, c : c + 512], wct, rhs[:, c : c + 512],
                        start=True, stop=True,
                    )
                ht = hbuf.tile([F, TILE], BF16, tag="ht")
                if ti in SPLIT_TILES:
                    act_relu(ht[:, : width // 2], hp[:, : width // 2])
                    dve_relu(ht[:, width // 2 : width], hp[:, width // 2 : width])
                elif ti in DVE_TILES:
                    dve_relu(ht[:, :width], hp[:, :width])
                else:
                    act_relu(ht[:, :width], hp[:, :width])
                hbufs.append((ht, width // CHUNK))

            def do_l4(ti):
                ht, n_ch = hbufs[ti]
                for j in range(n_ch):
                    c = state["chunk"]
                    s = c - state["bstart"]
                    if s == 0:
                        state["pp"] = ps_p.tile(
                            [F, SLOTS, OUT], F32, tag="pp", name="pp"
                        )
                    nc.tensor.matmul(
                        state["pp"][:, s, :],
                        ht[:, j * CHUNK : (j + 1) * CHUNK], w4at,
                        start=True, stop=True,
                    )
                    state["chunk"] = c + 1
                    if state["chunk"] in BATCH_ENDS:
                        c0 = state["bstart"]
                        n = state["chunk"] - c0
                        bi = state["bi"]
                        pts = ptb.tile([F, SLOTS, OUT], F32, tag="pts")
                        if BATCH_ENGINE[bi] == "dve":
                            nc.vector.tensor_copy(
                                pts[:, :n, :], state["pp"][:, :n, :]
                            )
                        else:
                            nc.scalar.activation(
                                pts[:, :n, :], state["pp"][:, :n, :], AF.Copy,
                            )
                        nc.sync.dma_start(
                            out=pt_ext[:, c0 : state["chunk"], :],
                            in_=pts[:, :n, :],
                        )
                        state["bstart"] = state["chunk"]
                        state["bi"] = bi + 1

            # prologue tile (hbufs[0]); steady tile t -> hbufs[t+1]
            bounds = []
            b = 0
            for w in XG_SIZES:
                bounds.append((b, b + w))
                b += w

            def xg_slice(c0, width):
                for g, (lo, hi) in enumerate(bounds):
                    if lo <= c0 and c0 + width <= hi:
                        return xg[g][:, c0 - lo : c0 - lo + width]
                raise AssertionError(f"tile [{c0}, {c0+width}) crosses groups")

            do_l1(-1, xg_slice(0, T0), T0)
            for t in range(N_TILES):
                do_l1(t, xg_slice(T0 + t * TILE, TILE), TILE)
                if t >= 1:
                    do_l4(t - 1)   # lag-2: chunks of tile t-2
            do_l4(N_TILES - 1)
            do_l4(N_TILES)

    nc.finalize()
    return nc


_CACHED = {}
TRACE = False
LAST = {}


def kernel(x, W1, b1, W2, b2, W3, b3, W4, b4, alpha):
    f64 = np.float64
    x2 = np.asarray(x, np.float32).reshape(N_ROWS, F)
    W1, b1, W2, b2, W3, b3, W4, b4, alpha = [
        np.asarray(a, f64) for a in (W1, b1, W2, b2, W3, b3, W4, b4, alpha)
    ]

    # fold the linear layers (exact in f64)
    Wc = W3 @ W2 @ W1
    bc = W3 @ (W2 @ b1 + b2) + b3
    alpha_safe = np.where(np.abs(alpha) < 1e-12, 1e-12, alpha)
    W4a = alpha_safe[:, None] * W4

    # pad rows to 8*25088 with zeros; pad rows dropped after the gather
    n_pad = ROWS_PAD - N_ROWS
    xp = np.concatenate([x2, np.zeros((n_pad, F), np.float32)], axis=0)
    xT = np.ascontiguousarray(xp.T).astype(ml_dtypes.float8_e4m3fn)

    has_bias = bool(np.any(bc != 0.0))
    key = ("nc", has_bias)
    if key not in _CACHED:
        _CACHED[key] = build_bass(has_bias)
    nc = _CACHED[key]

    wct_np = np.ascontiguousarray(Wc.T).astype(ml_dtypes.bfloat16)
    w4at_np = np.ascontiguousarray(W4a.T).astype(ml_dtypes.bfloat16)
    parts_list = [wct_np, w4at_np]
    if has_bias:
        parts_list.append(
            bc.reshape(F, 1).astype(np.float32).astype(ml_dtypes.bfloat16)
        )
    consts_np = np.ascontiguousarray(np.concatenate(parts_list, axis=1))

    bounds = np.cumsum((0,) + XG_SIZES)
    in_maps = []
    for c in range(N_CORES):
        shard = xT[:, c * RPC : (c + 1) * RPC]
        m = {"cst": consts_np}
        for i in range(len(XG_SIZES)):
            m[f"xg{i}"] = np.ascontiguousarray(shard[:, bounds[i] : bounds[i + 1]])
        in_maps.append(m)

    res = run_bass_kernel_spmd(
        nc, in_maps, core_ids=list(range(N_CORES)), trace=TRACE
    )
    LAST["res"] = res

    # gather pt: per core [F(part=row-in-chunk), N_CHUNKS, OUT]
    pts = np.stack([np.asarray(r["pt"], np.float32) for r in res.results])
    # rows order: (core, chunk, partition)
    pt = pts.transpose(0, 2, 1, 3).reshape(ROWS_PAD, OUT).astype(f64)
    pt = pt[:N_ROWS]

    # host softmax pooling in f64:  out_o = sum pt*e^pt / (alpha*sum e^pt) + b4
    m = pt.max(axis=0)
    e = np.exp(pt - m)
    den = e.sum(axis=0)
    num = (pt * e).sum(axis=0)
    out = num / (alpha_safe * den) + b4
    return out[None, :].astype(np.float32)


# revision 14
# speedup vs baseline: 1.2409x; 1.0144x over previous
"""Trainium2 Bass kernel for nn_AdaptivePoolingClassifier (8 NeuronCores).

Math: the reference MLP is linear up to its single ReLU, so W1..W3 fold
into one 128x128 matrix on the host:
    h   = relu(x @ Wc^T + bc)       Wc = W3 W2 W1 ; bc = W3(W2 b1+b2)+b3
    p   = h @ W4^T + b4
    out = sum_n p * softmax(alpha*p, axis=1)

Device computes pt = h @ (diag(alpha) W4)^T = alpha*(p - b4) for every
row (rows sharded 8 ways) and streams pt back to DRAM; the host finishes
the softmax pooling in f64 (num/den sums over rows) exactly as it
already finishes the fold / bias algebra.  The softmax weights are
invariant to the per-column constant alpha*b4 shift.

Device schedule (v5) — measured-engine-rate driven:
  - x host-transposed to [128(feat), rows] fp8e4 (halves HBM traffic;
    bf16 weights, mixed-dtype matmul, rel err ~1.8e-3).
  - All x DMAs issued UP FRONT from two queues (sync: cst+g0, gpsimd:
    g1..g4) into persistent SBUF tiles (24.5KB/partition) - no buffer
    recycling, no mid-kernel issue cost on busy queues.
  - PE warms up on garbage SBUF from t~1.5us (no data deps) so the
    DVFS ramp overlaps the initial DMA instead of following it.
  - Steady tiles of 1536 cols: 3x512 L1 matmuls (wct stationary) into
    a 3-bank psum tile, one whole-tile relu (ACT / DVE alternating,
    9:7 split matched to 1.33 vs 0.9 GHz effective rates) -> bf16 h,
    then 12 L4 chunk matmuls (h-chunk stationary [128,128], w4at
    moving 5 cols) whose ldweights pipeline back-to-back (~35ns/chunk
    measured dense vs ~100ns exposed).
  - pt accumulates in psum [128, 48, 5] batches; batch ends staggered
    (48, 96, 144, 180, 196) so copies+DMAs overlap the PE stream.
"""

import numpy as np
import ml_dtypes

from concourse import bacc, mybir, tile
from concourse.bass_utils import run_bass_kernel_spmd

N_CORES = 8
N_ROWS = 200000
F = 128
OUT = 5

ROWS_PAD = 200704            # 8 * 25088
RPC = ROWS_PAD // N_CORES    # rows per core = 25088
T0 = 512                     # prologue tile
TILE = 1024                  # steady tile (2 psum banks)
N_TILES = (RPC - T0) // TILE  # 24
CHUNK = 128
N_CHUNKS = RPC // CHUNK      # 196
SLOTS = 48                   # max pt chunks per psum batch
BATCH_ENDS = (48, 96, 144, 188, 196)
BATCH_ENGINE = ("act", "dve", "act", "dve", "dve")
# ramping x DMA groups: cst+xg0 on gpsimd (earliest queue), rest FIFO
# on sync; early groups small so the first tiles never starve
XG_SIZES = (T0, TILE, TILE, 2 * TILE, 3 * TILE, 4 * TILE, 5 * TILE,
            8 * TILE)
# tiles on the DVE relu path (rest on ACT); last two split across both
DVE_TILES = frozenset((1, 3, 5, 7, 9, 11, 13, 15, 17, 19, 21))
SPLIT_TILES = frozenset((22, 23))

F32 = mybir.dt.float32
BF16 = mybir.dt.bfloat16
FP8 = mybir.dt.float8e4
AF = mybir.ActivationFunctionType
ALU = mybir.AluOpType


def build_bass(has_bias=False):
    nc = bacc.Bacc()

    CONST_COLS = (F + OUT + 1) if has_bias else (F + OUT)
    cst_ext = nc.declare_dram_parameter(
        "cst", [F, CONST_COLS], BF16, isOutput=False
    )
    xg_ext = [
        nc.declare_dram_parameter(f"xg{i}", [F, w], FP8, isOutput=False)
        for i, w in enumerate(XG_SIZES)
    ]
    pt_ext = nc.declare_dram_parameter(
        "pt", [F, N_CHUNKS, OUT], F32, isOutput=True
    )

    with tile.TileContext(nc) as tc:
        with (
            tc.tile_pool(name="scratch", bufs=1) as scratch,
            tc.tile_pool(name="xin", bufs=1) as xin,
            tc.tile_pool(name="hbuf", bufs=4) as hbuf,
            tc.tile_pool(name="ptb", bufs=2) as ptb,
            tc.tile_pool(name="ps_h", bufs=3, space="PSUM") as ps_h,
            tc.tile_pool(name="ps_p", bufs=2, space="PSUM") as ps_p,
        ):
            # --- junk memsets first on gpsimd (earliest user queue) ------
            junk_w = scratch.tile([F, CHUNK], BF16)
            junk_x = scratch.tile([F, 512], FP8)
            junk_o = scratch.tile([F, 8], BF16)
            nc.gpsimd.memset(junk_w[:], 1.0)
            nc.gpsimd.memset(junk_x[:], 1.0)

            # --- upfront DMA issue, ALL on sync: per-queue FIFO means
            # cst/xg0 complete first at full aggregate DMA bandwidth ------
            cstt = scratch.tile([F, CONST_COLS], BF16)
            xg = [xin.tile([F, w], FP8, tag=f"xg{i}", name=f"xg{i}")
                  for i, w in enumerate(XG_SIZES)]
            nc.gpsimd.dma_start(out=cstt[:], in_=cst_ext[:])
            nc.gpsimd.dma_start(out=xg[0][:], in_=xg_ext[0][:])
            for i in range(1, len(XG_SIZES)):
                nc.sync.dma_start(out=xg[i][:], in_=xg_ext[i][:])

            wct = cstt[:, :F]
            w4at = cstt[:, F : F + OUT]
            bc = None
            if has_bias:
                bc = scratch.tile([F, 1], F32)

            # --- engine warmups (gated only on the vector memsets) -------
            pw = ps_h.tile([F, TILE], F32, tag="hp", name="pw")
            # preload ACT relu table during the DMA wait
            nc.scalar.activation(junk_o[:], junk_w[:, :8], AF.Relu)
            for r in range(4):
                nc.tensor.matmul(
                    pw[:, :512], junk_w, junk_x[:], start=True,
                    stop=True, skip_group_check=True,
                )
            if has_bias:
                nc.vector.tensor_copy(bc[:], cstt[:, F + OUT : F + OUT + 1])

            state = {"chunk": 0, "pp": None, "bstart": 0, "bi": 0}
            hbufs = []  # per tile: (htile, n_chunks)

            def act_relu(dst, src):
                if has_bias:
                    nc.scalar.activation(dst, src, AF.Relu, bias=bc[:], scale=1.0)
                else:
                    nc.scalar.activation(dst, src, AF.Relu)

            def dve_relu(dst, src):
                if has_bias:
                    nc.vector.tensor_scalar(dst, src, bc[:], 0.0, ALU.add, ALU.max)
                else:
                    nc.vector.tensor_scalar_max(dst, src, 0.0)

            def do_l1(ti, rhs, width):
                hp = ps_h.tile([F, TILE], F32, tag="hp", name="hp")
                for c in range(0, width, 512):
                    nc.tensor.matmul(
                        hp[:, c : c + 512], wct, rhs[:, c : c + 512],
                        start=True, stop=True,
                    )
                ht = hbuf.tile([F, TILE], BF16, tag="ht")
                if ti in SPLIT_TILES:
                    act_relu(ht[:, : width // 2], hp[:, : width // 2])
                    dve_relu(ht[:, width // 2 : width], hp[:, width // 2 : width])
                elif ti in DVE_TILES:
                    dve_relu(ht[:, :width], hp[:, :width])
                else:
                    act_relu(ht[:, :width], hp[:, :width])
                hbufs.append((ht, width // CHUNK))

            def do_l4(ti):
                ht, n_ch = hbufs[ti]
                for j in range(n_ch):
                    c = state["chunk"]
                    s = c - state["bstart"]
                    if s == 0:
                        state["pp"] = ps_p.tile(
                            [F, SLOTS, OUT], F32, tag="pp", name="pp"
                        )
                    nc.tensor.matmul(
                        state["pp"][:, s, :],
                        ht[:, j * CHUNK : (j + 1) * CHUNK], w4at,
                        start=True, stop=True,
                    )
                    state["chunk"] = c + 1
                    if state["chunk"] in BATCH_ENDS:
                        c0 = state["bstart"]
                        n = state["chunk"] - c0
                        bi = state["bi"]
                        pts = ptb.tile([F, SLOTS, OUT], F32, tag="pts")
                        if BATCH_ENGINE[bi] == "dve":
                            nc.vector.tensor_copy(
                                pts[:, :n, :], state["pp"][:, :n, :]
                            )
                        else:
                            nc.scalar.activation(
                                pts[:, :n, :], state["pp"][:, :n, :], AF.Copy,
                            )
                        nc.sync.dma_start(
                            out=pt_ext[:, c0 : state["chunk"], :],
                            in_=pts[:, :n, :],
                        )
                        state["bstart"] = state["chunk"]
                        state["bi"] = bi + 1

            # prologue tile (hbufs[0]); steady tile t -> hbufs[t+1]
            bounds = []
            b = 0
            for w in XG_SIZES:
                bounds.append((b, b + w))
                b += w

            def xg_slice(c0, width):
                for g, (lo, hi) in enumerate(bounds):
                    if lo <= c0 and c0 + width <= hi:
                        return xg[g][:, c0 - lo : c0 - lo + width]
                raise AssertionError(f"tile [{c0}, {c0+width}) crosses groups")

            do_l1(-1, xg_slice(0, T0), T0)
            for t in range(N_TILES):
                do_l1(t, xg_slice(T0 + t * TILE, TILE), TILE)
                if t == 1:
                    do_l4(0)            # prologue chunks
                elif t >= 3 and t % 2 == 1:
                    do_l4(t - 2)        # paired trains: tiles t-3, t-2
                    do_l4(t - 1)
            do_l4(N_TILES - 1)
            do_l4(N_TILES)

    nc.finalize()
    return nc


_CACHED = {}
TRACE = False
LAST = {}


def kernel(x, W1, b1, W2, b2, W3, b3, W4, b4, alpha):
    f64 = np.float64
    x2 = np.asarray(x, np.float32).reshape(N_ROWS, F)
    W1, b1, W2, b2, W3, b3, W4, b4, alpha = [
        np.asarray(a, f64) for a in (W1, b1, W2, b2, W3, b3, W4, b4, alpha)
    ]

    # fold the linear layers (exact in f64)
    Wc = W3 @ W2 @ W1
    bc = W3 @ (W2 @ b1 + b2) + b3
    alpha_safe = np.where(np.abs(alpha) < 1e-12, 1e-12, alpha)
    W4a = alpha_safe[:, None] * W4

    # pad rows to 8*25088 with zeros; pad rows dropped after the gather
    n_pad = ROWS_PAD - N_ROWS
    xp = np.concatenate([x2, np.zeros((n_pad, F), np.float32)], axis=0)
    xT = np.ascontiguousarray(xp.T).astype(ml_dtypes.float8_e4m3fn)

    has_bias = bool(np.any(bc != 0.0))
    key = ("nc", has_bias)
    if key not in _CACHED:
        _CACHED[key] = build_bass(has_bias)
    nc = _CACHED[key]

    wct_np = np.ascontiguousarray(Wc.T).astype(ml_dtypes.bfloat16)
    w4at_np = np.ascontiguousarray(W4a.T).astype(ml_dtypes.bfloat16)
    parts_list = [wct_np, w4at_np]
    if has_bias:
        parts_list.append(
            bc.reshape(F, 1).astype(np.float32).astype(ml_dtypes.bfloat16)
        )
    consts_np = np.ascontiguousarray(np.concatenate(parts_list, axis=1))

    bounds = np.cumsum((0,) + XG_SIZES)
    in_maps = []
    for c in range(N_CORES):
        shard = xT[:, c * RPC : (c + 1) * RPC]
        m = {"cst": consts_np}
        for i in range(len(XG_SIZES)):
            m[f"xg{i}"] = np.ascontiguousarray(shard[:, bounds[i] : bounds[i + 1]])
        in_maps.append(m)

    res = run_bass_kernel_spmd(
        nc, in_maps, core_ids=list(range(N_CORES)), trace=TRACE
    )
    LAST["res"] = res

    # gather pt: per core [F(part=row-in-chunk), N_CHUNKS, OUT]
    pts = np.stack([np.asarray(r["pt"], np.float32) for r in res.results])
    # rows order: (core, chunk, partition)
    pt = pts.transpose(0, 2, 1, 3).reshape(ROWS_PAD, OUT).astype(f64)
    pt = pt[:N_ROWS]

    # host softmax pooling in f64:  out_o = sum pt*e^pt / (alpha*sum e^pt) + b4
    m = pt.max(axis=0)
    e = np.exp(pt - m)
    den = e.sum(axis=0)
    num = (pt * e).sum(axis=0)
    out = num / (alpha_safe * den) + b4
    return out[None, :].astype(np.float32)
